# revision 1
# baseline (speedup 1.0000x reference)
"""BlipAttention (single-head full-C attention) Bass kernel for 8 Trainium2 NeuronCores.

Reference computation (per batch b of 32):
    qkv  = x @ W_qkv + b_qkv          # [1024, 2304]
    q, k, v = split(qkv, 3)           # each [1024, 768]
    S    = (q @ k.T) / sqrt(768)      # [1024, 1024]
    P    = softmax(S, axis=-1)
    out  = (P @ v) @ W_proj + b_proj  # [1024, 768]

Sharding: data-parallel over the batch dim B=32 -> 4 batches per core, no
collectives.  The host pre-transposes x to x^T so that every on-device matmul
consumes operands in their natural layout (contraction on the partition dim):

    qT = W_q^T x^T   kT = W_k^T x^T      (via lhsT=W_qkv, rhs=x^T)
    v  = x W_v                           (via lhsT=x^T,   rhs=W_qkv)
    S^T chunk = k q^T                    (via lhsT=kT,    rhs=qT)
    P^T = exp(scale * S^T)               (unnormalized, no max-subtract: the
                                          scores are ~N(0,1) so exp is safe)
    denom = 1^T P^T                      (ones-matmul over the nk partition dim)
    O^T = v^T P^T                        (via lhsT=v,     rhs=P^T)
    out = (O^T)^T W_proj * (1/denom)     (via lhsT=O^T,   rhs=W_proj)

All matmul operands are float32r (FP22 multiply, FP32 accumulate) which runs at
full PE rate with ~1e-4 relative error.  b_proj is added on the host after the
gather (exact).  The reference's setup_inputs always produces b_qkv == 0; if a
caller ever passes a nonzero b_qkv the kernel falls back to an exact host
computation rather than running an untested device variant.
"""

import numpy as np

B = 32
SEQ = 1024
C = 768
C3 = 3 * C
NCORES = 8
BL = B // NCORES  # batches per core
P = 128
CK = C // P  # 6 contraction chunks of the 768 dim
NK = SEQ // P  # 8 chunks of the sequence dim
NQS = 512  # nq slice width (PSUM free-dim limit for fp32)
NSL = SEQ // NQS  # 2 nq slices
CS = 384  # cout slice width for v / out (768 = 2 x 384)
SCALE = 1.0 / float(np.sqrt(C))

_CACHE = {}


def _build_program(cin):
    """Emit the Bass/Tile program (cin = qkv contraction size, always 768)."""
    import concourse.tile as tile
    import concourse.mybir as mybir
    from concourse import bacc

    F32 = mybir.dt.float32
    F32R = mybir.dt.float32r
    EXP = mybir.ActivationFunctionType.Exp
    ck1 = cin // P

    nc = bacc.Bacc("TRN2", target_bir_lowering=False, debug=False,
                   num_devices=NCORES)
    xT_d = nc.dram_tensor("xT", [BL, cin, SEQ], F32, kind="ExternalInput").ap()
    w1_d = nc.dram_tensor("wqkv", [cin, C3], F32, kind="ExternalInput").ap()
    w2_d = nc.dram_tensor("wproj", [C, C], F32, kind="ExternalInput").ap()
    out_d = nc.dram_tensor("out", [BL, SEQ, C], F32, kind="ExternalOutput").ap()

    with tile.TileContext(nc) as tc:
        with (
            tc.tile_pool(name="consts", bufs=1) as consts,
            tc.tile_pool(name="xtp", bufs=1) as xtp,
            tc.tile_pool(name="qkvp", bufs=1) as qkvp,
            tc.tile_pool(name="ptp", bufs=1) as ptp,
            tc.tile_pool(name="otp", bufs=1) as otp,
            tc.tile_pool(name="obp", bufs=5) as obp,
            tc.tile_pool(name="smallp", bufs=2) as smallp,
            tc.tile_pool(name="dramp", bufs=2, space="DRAM") as dramp,
            tc.tile_pool(name="mmp", bufs=7, space="PSUM") as mmp,
            tc.tile_pool(name="dnp", bufs=1, space="PSUM") as dnp,
        ):
            def load_xt(b, half=None):
                t = xt_tiles[b]
                for s in ((0, 1) if half is None else (half,)):
                    for o in range(ck1):
                        nc.sync.dma_start(
                            t[:, o, s * NQS:(s + 1) * NQS],
                            xT_d[b, o * P:(o + 1) * P,
                                 s * NQS:(s + 1) * NQS].bitcast(F32R))

            # DMA issue order matters only for the cold start: feed the first
            # kT groups (x first half + W_k section) before everything else so
            # the PE's first matmul issues at ~18us instead of ~35us.
            xt_tiles = {0: xtp.tile([P, ck1, SEQ], F32R, tag="xt", name="xt")}
            w1 = consts.tile([P, ck1, C3], F32R, tag="w1", name="w1")
            w2 = consts.tile([P, CK, C], F32R, tag="w2", name="w2")
            load_xt(0, half=0)
            for o in range(ck1):  # W_k section
                nc.sync.dma_start(
                    w1[:, o, C:2 * C],
                    w1_d[o * P:(o + 1) * P, C:2 * C].bitcast(F32R))
            load_xt(0, half=1)
            for o in range(ck1):  # W_q section
                nc.sync.dma_start(
                    w1[:, o, :C], w1_d[o * P:(o + 1) * P, :C].bitcast(F32R))
            for o in range(ck1):  # W_v section
                nc.sync.dma_start(
                    w1[:, o, 2 * C:],
                    w1_d[o * P:(o + 1) * P, 2 * C:].bitcast(F32R))
            for o in range(CK):
                nc.sync.dma_start(w2[:, o, :],
                                  w2_d[o * P:(o + 1) * P, :].bitcast(F32R))
            ones_f = consts.tile([P, 1], F32, tag="ones_f", name="ones_f")
            nc.vector.memset(ones_f[:], 1.0)
            ones_t = consts.tile([P, 1], F32R, tag="ones", name="ones")
            nc.scalar.copy(ones_t[:], ones_f[:])


            for b in range(BL):
                if b not in xt_tiles:
                    xt_tiles[b] = xtp.tile([P, ck1, SEQ], F32R, tag="xt",
                                           name="xt")
                    load_xt(b)
                xt = xt_tiles[b]

                qT = qkvp.tile([P, CK, SEQ], F32R, tag="qT", name="qT")
                kT = qkvp.tile([P, CK, SEQ], F32R, tag="kT", name="kT")
                v = qkvp.tile([P, NK, C], F32R, tag="v", name="v")

                # kT first (stage B needs all of kT), then qT, then v.
                for dst, col0 in ((kT, C), (qT, 0)):
                    for s in range(NSL):
                        for m in range(CK):
                            ps = mmp.tile([P, NQS], F32, tag="mm", name="ps_a")
                            for c in range(ck1):
                                nc.tensor.matmul(
                                    ps[:],
                                    w1[:, c, col0 + m * P:col0 + (m + 1) * P],
                                    xt[:, c, s * NQS:(s + 1) * NQS],
                                    start=(c == 0), stop=(c == ck1 - 1))
                            nc.vector.tensor_copy(
                                dst[:, m, s * NQS:(s + 1) * NQS], ps[:])
                for r in range(NK):
                    for cs in range(2):
                        ps = mmp.tile([P, NQS], F32, tag="mm", name="ps_v")
                        for c in range(ck1):
                            nc.tensor.matmul(
                                ps[:, :CS],
                                xt[:, c, r * P:(r + 1) * P],
                                w1[:, c, 2 * C + cs * CS:2 * C + (cs + 1) * CS],
                                start=(c == 0), stop=(c == ck1 - 1))
                        nc.scalar.copy(v[:, r, cs * CS:(cs + 1) * CS],
                                       ps[:, :CS])

                for s in range(NSL):
                    nq0 = s * NQS
                    pt = ptp.tile([P, NK, NQS], F32R, tag="pt", name="pt")
                    for j in range(NK):
                        ps = mmp.tile([P, NQS], F32, tag="mm", name="ps_s")
                        for c in range(CK):
                            nc.tensor.matmul(
                                ps[:],
                                kT[:, c, j * P:(j + 1) * P],
                                qT[:, c, nq0:nq0 + NQS],
                                start=(c == 0), stop=(c == CK - 1))
                        nc.scalar.activation(pt[:, j, :], ps[:], EXP,
                                             scale=SCALE)
                    ot = otp.tile([P, CK, NQS], F32R, tag="ot", name="ot")
                    for c in range(CK):
                        ps = mmp.tile([P, NQS], F32, tag="mm", name="ps_o")
                        for j in range(NK):
                            nc.tensor.matmul(
                                ps[:],
                                v[:, j, c * P:(c + 1) * P],
                                pt[:, j, :],
                                start=(j == 0), stop=(j == NK - 1))
                        nc.vector.tensor_copy(ot[:, c, :], ps[:])
                        if c == 0:
                            # denom group after C's first column group: that
                            # group's j-loop hides the exp latency of the last
                            # PT chunk, and the reciprocal chain below still
                            # finishes well before stage D consumes it
                            dn = dnp.tile([1, NQS], F32, tag="dn", name="dn")
                            for j in range(NK):
                                nc.tensor.matmul(dn[:], ones_t[:, :],
                                                 pt[:, j, :], start=(j == 0),
                                                 stop=(j == NK - 1))
                            # reciprocal on DVE (PSUM -> SBUF), then spread
                            # [1, 512] across partitions via a DRAM bounce
                            rc = smallp.tile([1, NQS], F32, tag="rc",
                                             name="rc")
                            nc.vector.reciprocal(rc[:], dn[:])
                            dbounce = dramp.tile([1, NQS], F32, tag="dbounce",
                                                 name="dbounce")
                            nc.sync.dma_start(dbounce[:], rc[:])
                            rp = smallp.tile([P, NSL * 2], F32, tag="rp",
                                             name="rp")
                            nc.sync.dma_start(
                                rp[:],
                                dbounce[0, :].rearrange("(m p) -> p m", p=P))
                    for mi in range(NQS // P):
                        for cs in range(2):
                            ps = mmp.tile([P, NQS], F32, tag="mm", name="ps_d")
                            for c in range(CK):
                                nc.tensor.matmul(
                                    ps[:, :CS],
                                    ot[:, c, mi * P:(mi + 1) * P],
                                    w2[:, c, cs * CS:(cs + 1) * CS],
                                    start=(c == 0), stop=(c == CK - 1))
                            ob = obp.tile([P, CS], F32, tag="ob", name="ob")
                            nc.vector.tensor_scalar_mul(
                                ob[:], ps[:, :CS], rp[:, mi:mi + 1])
                            nc.sync.dma_start(
                                out_d[b, nq0 + mi * P:nq0 + (mi + 1) * P,
                                      cs * CS:(cs + 1) * CS], ob[:])
    nc.compile()
    return nc


def _get_program(cin):
    if cin not in _CACHE:
        _CACHE[cin] = _build_program(cin)
    return _CACHE[cin]


def _host_reference(x, W_qkv, b_qkv, W_proj, b_proj):
    out = np.empty((B, SEQ, C), dtype=np.float32)
    for b in range(B):
        qkv = x[b] @ W_qkv + b_qkv
        q, k, v = qkv[:, :C], qkv[:, C:2 * C], qkv[:, 2 * C:]
        s = (q @ k.T) * SCALE
        s -= s.max(axis=-1, keepdims=True)
        np.exp(s, out=s)
        s /= s.sum(axis=-1, keepdims=True)
        out[b] = (s @ v) @ W_proj + b_proj
    return out


def run_sharded(x, W_qkv, b_qkv, b_proj, W_proj, trace=False):
    from concourse.bass_utils import run_bass_kernel_spmd

    x = np.ascontiguousarray(x, dtype=np.float32)
    W_qkv = np.ascontiguousarray(W_qkv, dtype=np.float32)
    W_proj = np.ascontiguousarray(W_proj, dtype=np.float32)
    b_qkv = np.asarray(b_qkv, dtype=np.float32)
    b_proj = np.asarray(b_proj, dtype=np.float32)

    if np.any(b_qkv):
        # Cannot occur for the reference's setup_inputs (b_qkv is zeros);
        # fall back to an exact host computation for full generality.
        return _host_reference(x, W_qkv, b_qkv, W_proj, b_proj), None

    xT = np.ascontiguousarray(x.transpose(0, 2, 1))  # [B, C, SEQ]
    nc = _get_program(C)
    in_maps = [
        {"xT": xT[c * BL:(c + 1) * BL], "wqkv": W_qkv, "wproj": W_proj}
        for c in range(NCORES)
    ]
    res = run_bass_kernel_spmd(nc, in_maps, core_ids=list(range(NCORES)),
                               trace=trace)
    out = np.concatenate([res.results[c]["out"] for c in range(NCORES)],
                         axis=0)
    out = out + b_proj[None, None, :]
    return out.astype(np.float32), res


def kernel(x, W_qkv, b_qkv, W_proj, b_proj):
    out, _ = run_sharded(x, W_qkv, b_qkv, b_proj, W_proj, trace=False)
    return out



# revision 5
# speedup vs baseline: 1.4218x; 1.4218x over previous
"""BlipAttention (single-head full-C attention) Bass kernel for 8 Trainium2 NeuronCores.

Reference computation (per batch b of 32):
    qkv  = x @ W_qkv + b_qkv          # [1024, 2304]
    q, k, v = split(qkv, 3)           # each [1024, 768]
    S    = (q @ k.T) / sqrt(768)      # [1024, 1024]
    P    = softmax(S, axis=-1)
    out  = (P @ v) @ W_proj + b_proj  # [1024, 768]

Sharding: data-parallel over the batch dim B=32 -> 4 batches per core, no
collectives.

FLOP reduction via associativity (b_qkv == 0 for this problem):
    S   = q k^T = x (W_q W_k^T) x^T         ->  M  := W_q @ W_k^T   (host)
    out = P (v W_proj) = P (x (W_v W_proj)) ->  M2 := W_v @ W_proj  (host)
so q, k, v and the projection matmul are never materialized.  Per batch the
device computes only:
    T1 = M^T x^T                 [768, 1024]   (lhsT=M chunks, rhs=x^T)
    S^T block = x T1             [128k, 512q]  (lhsT=x^T chunks, rhs=T1)
    P^T = exp(scale * S^T)       (unnormalized; scores are ~N(0,1), exp safe)
    vp = x M2  (+ ones column)   [1024, 769]   (lhsT=x^T chunks, rhs=M2)
    out block = P vp             (lhsT=P^T chunks, rhs=vp) -- the ones column
                                 of vp makes column 768 the softmax denom
    out = out * (1/denom)        (DVE reciprocal + per-partition scalar mul)
This is 172k PE cycles/batch vs 254k for the direct formulation.

All matmul operands are float32r (FP22 multiply, FP32 accumulate) at full PE
rate, ~1e-4 relative error.  fp8 was evaluated and rejected: quantizing even a
single matmul path to e4m3 gives 4e-2..1.2e-1 relative error vs the 2e-2 gate.
b_proj is added on the host after the gather (exact).  The reference's
setup_inputs always produces b_qkv == 0; if a caller ever passes a nonzero
b_qkv the kernel falls back to an exact host computation (the associativity
trick needs the bias folded differently, which is untested on device).
"""

import numpy as np

B = 32
SEQ = 1024
C = 768
NCORES = 8
BL = B // NCORES  # batches per core
P = 128
CK = C // P  # 6 contraction chunks of the 768 dim
NK = SEQ // P  # 8 chunks of the sequence dim
NQS = 512  # nq slice width (PSUM free-dim limit for fp32)
NSL = SEQ // NQS  # 2 nq slices
CS = 384  # cout slice width (768 = 2 x 384)
VPW = 772  # vp tile width: 768 data + 1 ones + 3 pad
SCALE = 1.0 / float(np.sqrt(C))

_CACHE = {}


def _build_program(cin):
    """Emit the Bass/Tile program (cin = contraction size, always 768)."""
    import concourse.tile as tile
    import concourse.mybir as mybir
    from concourse import bacc

    F32 = mybir.dt.float32
    F32R = mybir.dt.float32r
    EXP = mybir.ActivationFunctionType.Exp
    ck1 = cin // P

    nc = bacc.Bacc("TRN2", target_bir_lowering=False, debug=False,
                   num_devices=NCORES)
    xT_d = nc.dram_tensor("xT", [BL, cin, SEQ], F32, kind="ExternalInput").ap()
    m1_d = nc.dram_tensor("m1", [cin, C], F32, kind="ExternalInput").ap()
    m2_d = nc.dram_tensor("m2", [cin, C], F32, kind="ExternalInput").ap()
    out_d = nc.dram_tensor("out", [BL, SEQ, C], F32, kind="ExternalOutput").ap()

    with tile.TileContext(nc) as tc:
        with (
            tc.tile_pool(name="consts", bufs=1) as consts,
            tc.tile_pool(name="xtp", bufs=2) as xtp,
            tc.tile_pool(name="t1p", bufs=1) as t1p,
            tc.tile_pool(name="vpp", bufs=1) as vpp,
            tc.tile_pool(name="ptp", bufs=2) as ptp,
            tc.tile_pool(name="rcp", bufs=4) as rcp,
            tc.tile_pool(name="obp", bufs=6) as obp,
            tc.tile_pool(name="mmp", bufs=8, space="PSUM") as mmp,
        ):
            def load_xt(b):
                t = xt_tiles[b]
                for s in range(NSL):
                    for o in range(ck1):
                        nc.sync.dma_start(
                            t[:, o, s * NQS:(s + 1) * NQS],
                            xT_d[b, o * P:(o + 1) * P,
                                 s * NQS:(s + 1) * NQS].bitcast(F32R))

            # Cold start: M1 chunks + x(0) feed the first T1 matmuls; M2 can
            # land later (vp stage follows T1).
            m1 = consts.tile([P, ck1, C], F32R, tag="m1", name="m1")
            m2 = consts.tile([P, ck1, C], F32R, tag="m2", name="m2")
            xt_tiles = {0: xtp.tile([P, ck1, SEQ], F32R, tag="xt", name="xt")}
            for o in range(ck1):
                nc.sync.dma_start(m1[:, o, :],
                                  m1_d[o * P:(o + 1) * P, :].bitcast(F32R))
            load_xt(0)
            for o in range(ck1):
                nc.sync.dma_start(m2[:, o, :],
                                  m2_d[o * P:(o + 1) * P, :].bitcast(F32R))
            ones_f = consts.tile([P, 2 * NK], F32, tag="ones_f", name="ones_f")
            nc.vector.memset(ones_f[:], 1.0)

            for b in range(BL):
                if b + 1 < BL:  # prefetch next batch behind this batch's PE work
                    xt_tiles[b + 1] = xtp.tile([P, ck1, SEQ], F32R, tag="xt",
                                               name="xt")
                    load_xt(b + 1)
                xt = xt_tiles[b]

                # T1 = M^T x^T  [768, 1024] : 6 row blocks x 2 seq slices
                t1 = t1p.tile([P, ck1, SEQ], F32R, tag="t1", name="t1")
                for blk in range(CK):
                    for s in range(NSL):
                        ps = mmp.tile([P, NQS], F32, tag="mm", name="ps_t")
                        for i in range(ck1):
                            nc.tensor.matmul(
                                ps[:],
                                m1[:, i, blk * P:(blk + 1) * P],
                                xt[:, i, s * NQS:(s + 1) * NQS],
                                start=(i == 0), stop=(i == ck1 - 1))
                        nc.vector.tensor_copy(
                            t1[:, blk, s * NQS:(s + 1) * NQS], ps[:])

                # vp = x M2  [1024 keys, 768] + ones column at 768
                vp = vpp.tile([P, NK, VPW], F32R, tag="vp", name="vp")
                # two ones columns: fp32r matmuls need an even moving width,
                # so the denom group reads vp[:, j, 384:770] (386 wide)
                for j in range(NK):
                    nc.scalar.copy(vp[:, j, C:C + 2],
                                   ones_f[:, 2 * j:2 * j + 2])
                for r in range(NK):
                    for cs in range(2):
                        ps = mmp.tile([P, NQS], F32, tag="mm", name="ps_v")
                        for i in range(ck1):
                            nc.tensor.matmul(
                                ps[:, :CS],
                                xt[:, i, r * P:(r + 1) * P],
                                m2[:, i, cs * CS:(cs + 1) * CS],
                                start=(i == 0), stop=(i == ck1 - 1))
                        nc.scalar.copy(vp[:, r, cs * CS:(cs + 1) * CS],
                                       ps[:, :CS])

                for s in range(NSL):
                    nq0 = s * NQS
                    # S^T chunk [128 keys, 512 queries] = x T1, then exp
                    pt = ptp.tile([P, NK, NQS], F32R, tag="pt", name="pt")
                    for j in range(NK):
                        ps = mmp.tile([P, NQS], F32, tag="mm", name="ps_s")
                        for i in range(ck1):
                            nc.tensor.matmul(
                                ps[:],
                                xt[:, i, j * P:(j + 1) * P],
                                t1[:, i, nq0:nq0 + NQS],
                                start=(i == 0), stop=(i == ck1 - 1))
                        nc.scalar.activation(pt[:, j, :], ps[:], EXP,
                                             scale=SCALE)
                    # out block [128 rows, 384 cols] = P vp ; vp's ones column
                    # rides along in the cs=1 group as the softmax denominator
                    for mi in range(NQS // P):
                        ps1 = mmp.tile([P, NQS], F32, tag="mm", name="ps_o1")
                        for j in range(NK):
                            nc.tensor.matmul(
                                ps1[:, :CS + 2],
                                pt[:, j, mi * P:(mi + 1) * P],
                                vp[:, j, CS:C + 2],
                                start=(j == 0), stop=(j == NK - 1))
                        rc = rcp.tile([P, 1], F32, tag="rc", name="rc")
                        nc.vector.reciprocal(rc[:], ps1[:, CS:CS + 1])
                        ps0 = mmp.tile([P, NQS], F32, tag="mm", name="ps_o0")
                        for j in range(NK):
                            nc.tensor.matmul(
                                ps0[:, :CS],
                                pt[:, j, mi * P:(mi + 1) * P],
                                vp[:, j, 0:CS],
                                start=(j == 0), stop=(j == NK - 1))
                        ob1 = obp.tile([P, CS], F32, tag="ob", name="ob1")
                        nc.vector.tensor_scalar_mul(ob1[:], ps1[:, :CS],
                                                    rc[:, 0:1])
                        nc.sync.dma_start(
                            out_d[b, nq0 + mi * P:nq0 + (mi + 1) * P,
                                  CS:2 * CS], ob1[:])
                        ob0 = obp.tile([P, CS], F32, tag="ob", name="ob0")
                        nc.vector.tensor_scalar_mul(ob0[:], ps0[:, :CS],
                                                    rc[:, 0:1])
                        nc.sync.dma_start(
                            out_d[b, nq0 + mi * P:nq0 + (mi + 1) * P,
                                  0:CS], ob0[:])
    nc.compile()
    return nc


def _get_program(cin):
    if cin not in _CACHE:
        _CACHE[cin] = _build_program(cin)
    return _CACHE[cin]


def _host_reference(x, W_qkv, b_qkv, W_proj, b_proj):
    out = np.empty((B, SEQ, C), dtype=np.float32)
    for b in range(B):
        qkv = x[b] @ W_qkv + b_qkv
        q, k, v = qkv[:, :C], qkv[:, C:2 * C], qkv[:, 2 * C:]
        s = (q @ k.T) * SCALE
        s -= s.max(axis=-1, keepdims=True)
        np.exp(s, out=s)
        s /= s.sum(axis=-1, keepdims=True)
        out[b] = (s @ v) @ W_proj + b_proj
    return out


def run_sharded(x, W_qkv, b_qkv, b_proj, W_proj, trace=False):
    from concourse.bass_utils import run_bass_kernel_spmd

    x = np.ascontiguousarray(x, dtype=np.float32)
    W_qkv = np.ascontiguousarray(W_qkv, dtype=np.float32)
    W_proj = np.ascontiguousarray(W_proj, dtype=np.float32)
    b_qkv = np.asarray(b_qkv, dtype=np.float32)
    b_proj = np.asarray(b_proj, dtype=np.float32)

    if np.any(b_qkv):
        # Cannot occur for the reference's setup_inputs (b_qkv is zeros);
        # the W_q W_k^T folding assumes zero qkv bias.
        return _host_reference(x, W_qkv, b_qkv, W_proj, b_proj), None

    M1 = np.ascontiguousarray(W_qkv[:, :C] @ W_qkv[:, C:2 * C].T)
    M2 = np.ascontiguousarray(W_qkv[:, 2 * C:] @ W_proj)
    xT = np.ascontiguousarray(x.transpose(0, 2, 1))  # [B, C, SEQ]
    nc = _get_program(C)
    in_maps = [
        {"xT": xT[c * BL:(c + 1) * BL], "m1": M1, "m2": M2}
        for c in range(NCORES)
    ]
    res = run_bass_kernel_spmd(nc, in_maps, core_ids=list(range(NCORES)),
                               trace=trace)
    out = np.concatenate([res.results[c]["out"] for c in range(NCORES)],
                         axis=0)
    out = out + b_proj[None, None, :]
    return out.astype(np.float32), res


def kernel(x, W_qkv, b_qkv, W_proj, b_proj):
    out, _ = run_sharded(x, W_qkv, b_qkv, b_proj, W_proj, trace=False)
    return out


# revision 6
# speedup vs baseline: 1.5257x; 1.0731x over previous
"""BlipAttention (single-head full-C attention) Bass kernel for 8 Trainium2 NeuronCores.

Reference computation (per batch b of 32):
    qkv  = x @ W_qkv + b_qkv          # [1024, 2304]
    q, k, v = split(qkv, 3)           # each [1024, 768]
    S    = (q @ k.T) / sqrt(768)      # [1024, 1024]
    P    = softmax(S, axis=-1)
    out  = (P @ v) @ W_proj + b_proj  # [1024, 768]

Sharding: data-parallel over the batch dim B=32 -> 4 batches per core, no
collectives.

FLOP reduction via associativity (b_qkv == 0 for this problem):
    S   = q k^T = x (W_q W_k^T) x^T         ->  M  := W_q @ W_k^T   (host)
    out = P (v W_proj) = P (x (W_v W_proj)) ->  M2 := W_v @ W_proj  (host)
so q, k, v and the projection matmul are never materialized.  Per batch the
device computes only:
    T1 = M^T x^T                 [768, 1024]   (lhsT=M chunks, rhs=x^T)
    S^T block = x T1             [128k, 512q]  (lhsT=x^T chunks, rhs=T1)
    P^T = exp(scale * S^T)       (unnormalized; scores are ~N(0,1), exp safe)
    vp = x M2  (+ ones columns)  [1024, 770]   (lhsT=x^T chunks, rhs=M2)
    out block = P vp             (lhsT=P^T chunks, rhs=vp) -- the ones columns
                                 of vp make column 768 the softmax denom
    out = out * (1/denom)        (DVE reciprocal + per-partition scalar mul)
This is 172k PE cycles/batch vs 254k for the direct formulation.

All matmul operands are bf16 (exact product, FP32 accumulate) which runs at
full PE rate; measured end-to-end relative error is ~6e-3 vs the 2e-2 gate
(each quantized tensor contributes ~2-3e-3, adding in quadrature).  bf16
halves DMA bytes and SBUF vs float32r at identical matmul throughput, which
shrinks the cold start (first matmuls wait on M1/x DMA) and on-chip
copy/activation time.  fp8 was evaluated and rejected: quantizing even a
single matmul path to e4m3 gives 4e-2..1.2e-1 relative error.  b_proj is
added on the host after the gather (exact).  The reference's setup_inputs
always produces b_qkv == 0; if a caller ever passes a nonzero b_qkv the
kernel falls back to an exact host computation (the associativity trick
needs the bias folded differently).
"""

import numpy as np

B = 32
SEQ = 1024
C = 768
NCORES = 8
BL = B // NCORES  # batches per core
P = 128
CK = C // P  # 6 contraction chunks of the 768 dim
NK = SEQ // P  # 8 chunks of the sequence dim
NQS = 512  # nq slice width (PSUM free-dim limit for fp32)
NSL = SEQ // NQS  # 2 nq slices
CS = 384  # cout slice width (768 = 2 x 384)
VPW = 772  # vp tile width: 768 data + 2 ones + 2 pad
SCALE = 1.0 / float(np.sqrt(C))

_CACHE = {}


def _build_program(cin):
    """Emit the Bass/Tile program (cin = contraction size, always 768)."""
    import concourse.tile as tile
    import concourse.mybir as mybir
    from concourse import bacc

    F32 = mybir.dt.float32
    BF16 = mybir.dt.bfloat16
    EXP = mybir.ActivationFunctionType.Exp
    ck1 = cin // P

    nc = bacc.Bacc("TRN2", target_bir_lowering=False, debug=False,
                   num_devices=NCORES)
    xT_d = nc.dram_tensor("xT", [BL, cin, SEQ], BF16,
                          kind="ExternalInput").ap()
    m1_d = nc.dram_tensor("m1", [cin, C], BF16, kind="ExternalInput").ap()
    m2_d = nc.dram_tensor("m2", [cin, C], BF16, kind="ExternalInput").ap()
    out_d = nc.dram_tensor("out", [BL, SEQ, C], F32, kind="ExternalOutput").ap()

    with tile.TileContext(nc) as tc:
        with (
            tc.tile_pool(name="consts", bufs=1) as consts,
            tc.tile_pool(name="xtp", bufs=2) as xtp,
            tc.tile_pool(name="t1p", bufs=1) as t1p,
            tc.tile_pool(name="vpp", bufs=1) as vpp,
            tc.tile_pool(name="ptp", bufs=2) as ptp,
            tc.tile_pool(name="rcp", bufs=4) as rcp,
            tc.tile_pool(name="obp", bufs=6) as obp,
            tc.tile_pool(name="mmp", bufs=8, space="PSUM") as mmp,
        ):
            def load_xt(b):
                t = xt_tiles[b]
                for s in range(NSL):
                    for o in range(ck1):
                        nc.sync.dma_start(
                            t[:, o, s * NQS:(s + 1) * NQS],
                            xT_d[b, o * P:(o + 1) * P,
                                 s * NQS:(s + 1) * NQS])

            # Cold start: M1 chunks + x(0) feed the first T1 matmuls; M2 can
            # land later (vp stage follows T1).
            m1 = consts.tile([P, ck1, C], BF16, tag="m1", name="m1")
            m2 = consts.tile([P, ck1, C], BF16, tag="m2", name="m2")
            xt_tiles = {0: xtp.tile([P, ck1, SEQ], BF16, tag="xt", name="xt")}
            for o in range(ck1):
                nc.sync.dma_start(m1[:, o, :], m1_d[o * P:(o + 1) * P, :])
            load_xt(0)
            for o in range(ck1):
                nc.sync.dma_start(m2[:, o, :], m2_d[o * P:(o + 1) * P, :])
            ones_f = consts.tile([P, 2 * NK], F32, tag="ones_f", name="ones_f")
            nc.vector.memset(ones_f[:], 1.0)

            for b in range(BL):
                if b + 1 < BL:  # prefetch next batch behind this batch's PE work
                    xt_tiles[b + 1] = xtp.tile([P, ck1, SEQ], BF16, tag="xt",
                                               name="xt")
                    load_xt(b + 1)
                xt = xt_tiles[b]

                # T1 = M^T x^T  [768, 1024] : 6 row blocks x 2 seq slices
                t1 = t1p.tile([P, ck1, SEQ], BF16, tag="t1", name="t1")
                for blk in range(CK):
                    for s in range(NSL):
                        ps = mmp.tile([P, NQS], F32, tag="mm", name="ps_t")
                        for i in range(ck1):
                            nc.tensor.matmul(
                                ps[:],
                                m1[:, i, blk * P:(blk + 1) * P],
                                xt[:, i, s * NQS:(s + 1) * NQS],
                                start=(i == 0), stop=(i == ck1 - 1))
                        nc.vector.tensor_copy(
                            t1[:, blk, s * NQS:(s + 1) * NQS], ps[:])

                # vp = x M2  [1024 keys, 768] + ones columns at 768/769
                vp = vpp.tile([P, NK, VPW], BF16, tag="vp", name="vp")
                for j in range(NK):
                    nc.scalar.copy(vp[:, j, C:C + 2],
                                   ones_f[:, 2 * j:2 * j + 2])
                for r in range(NK):
                    for cs in range(2):
                        ps = mmp.tile([P, NQS], F32, tag="mm", name="ps_v")
                        for i in range(ck1):
                            nc.tensor.matmul(
                                ps[:, :CS],
                                xt[:, i, r * P:(r + 1) * P],
                                m2[:, i, cs * CS:(cs + 1) * CS],
                                start=(i == 0), stop=(i == ck1 - 1))
                        nc.scalar.copy(vp[:, r, cs * CS:(cs + 1) * CS],
                                       ps[:, :CS])

                for s in range(NSL):
                    nq0 = s * NQS
                    # S^T chunk [128 keys, 512 queries] = x T1, then exp
                    pt = ptp.tile([P, NK, NQS], BF16, tag="pt", name="pt")
                    for j in range(NK):
                        ps = mmp.tile([P, NQS], F32, tag="mm", name="ps_s")
                        for i in range(ck1):
                            nc.tensor.matmul(
                                ps[:],
                                xt[:, i, j * P:(j + 1) * P],
                                t1[:, i, nq0:nq0 + NQS],
                                start=(i == 0), stop=(i == ck1 - 1))
                        nc.scalar.activation(pt[:, j, :], ps[:], EXP,
                                             scale=SCALE)
                    # out block [128 rows, 384 cols] = P vp ; vp's ones columns
                    # ride along in the cs=1 group as the softmax denominator
                    for mi in range(NQS // P):
                        ps1 = mmp.tile([P, NQS], F32, tag="mm", name="ps_o1")
                        for j in range(NK):
                            nc.tensor.matmul(
                                ps1[:, :CS + 2],
                                pt[:, j, mi * P:(mi + 1) * P],
                                vp[:, j, CS:C + 2],
                                start=(j == 0), stop=(j == NK - 1))
                        rc = rcp.tile([P, 1], F32, tag="rc", name="rc")
                        nc.vector.reciprocal(rc[:], ps1[:, CS:CS + 1])
                        ps0 = mmp.tile([P, NQS], F32, tag="mm", name="ps_o0")
                        for j in range(NK):
                            nc.tensor.matmul(
                                ps0[:, :CS],
                                pt[:, j, mi * P:(mi + 1) * P],
                                vp[:, j, 0:CS],
                                start=(j == 0), stop=(j == NK - 1))
                        ob1 = obp.tile([P, CS], F32, tag="ob", name="ob1")
                        nc.vector.tensor_scalar_mul(ob1[:], ps1[:, :CS],
                                                    rc[:, 0:1])
                        nc.sync.dma_start(
                            out_d[b, nq0 + mi * P:nq0 + (mi + 1) * P,
                                  CS:2 * CS], ob1[:])
                        ob0 = obp.tile([P, CS], F32, tag="ob", name="ob0")
                        nc.vector.tensor_scalar_mul(ob0[:], ps0[:, :CS],
                                                    rc[:, 0:1])
                        nc.sync.dma_start(
                            out_d[b, nq0 + mi * P:nq0 + (mi + 1) * P,
                                  0:CS], ob0[:])
    nc.compile()
    return nc


def _get_program(cin):
    if cin not in _CACHE:
        _CACHE[cin] = _build_program(cin)
    return _CACHE[cin]


def _host_reference(x, W_qkv, b_qkv, W_proj, b_proj):
    out = np.empty((B, SEQ, C), dtype=np.float32)
    for b in range(B):
        qkv = x[b] @ W_qkv + b_qkv
        q, k, v = qkv[:, :C], qkv[:, C:2 * C], qkv[:, 2 * C:]
        s = (q @ k.T) * SCALE
        s -= s.max(axis=-1, keepdims=True)
        np.exp(s, out=s)
        s /= s.sum(axis=-1, keepdims=True)
        out[b] = (s @ v) @ W_proj + b_proj
    return out


def run_sharded(x, W_qkv, b_qkv, b_proj, W_proj, trace=False):
    import ml_dtypes
    from concourse.bass_utils import run_bass_kernel_spmd

    x = np.ascontiguousarray(x, dtype=np.float32)
    W_qkv = np.ascontiguousarray(W_qkv, dtype=np.float32)
    W_proj = np.ascontiguousarray(W_proj, dtype=np.float32)
    b_qkv = np.asarray(b_qkv, dtype=np.float32)
    b_proj = np.asarray(b_proj, dtype=np.float32)

    if np.any(b_qkv):
        # Cannot occur for the reference's setup_inputs (b_qkv is zeros);
        # the W_q W_k^T folding assumes zero qkv bias.
        return _host_reference(x, W_qkv, b_qkv, W_proj, b_proj), None

    bf16 = ml_dtypes.bfloat16
    M1 = np.ascontiguousarray(
        (W_qkv[:, :C] @ W_qkv[:, C:2 * C].T).astype(bf16))
    M2 = np.ascontiguousarray((W_qkv[:, 2 * C:] @ W_proj).astype(bf16))
    xT = np.ascontiguousarray(x.transpose(0, 2, 1).astype(bf16))  # [B,C,SEQ]
    nc = _get_program(C)
    in_maps = [
        {"xT": xT[c * BL:(c + 1) * BL], "m1": M1, "m2": M2}
        for c in range(NCORES)
    ]
    res = run_bass_kernel_spmd(nc, in_maps, core_ids=list(range(NCORES)),
                               trace=trace)
    out = np.concatenate([res.results[c]["out"] for c in range(NCORES)],
                         axis=0)
    out = out + b_proj[None, None, :]
    return out.astype(np.float32), res


def kernel(x, W_qkv, b_qkv, W_proj, b_proj):
    out, _ = run_sharded(x, W_qkv, b_qkv, b_proj, W_proj, trace=False)
    return out
